# revision 41
# baseline (speedup 1.0000x reference)
"""MoE FFN (top-2 routing, 8 experts) on 8 Trainium2 NeuronCores.

Strategy (expert parallelism, per the sharding hint):
  - Host computes router logits / top-2 / softmax (tiny: T x E) and
    dispatches tokens: expert e's tokens are gathered into a padded
    [H, C] batch for core e (C = common capacity, multiple of 128).
  - Core e runs the dense FFN for its expert on its gathered tokens:
        yT = wt ⊙ ( GELU_tanh(x @ W1 + b1) @ W2 + b2 )^T
    computed fully transposed ([F,C] then [H,C]) so both matmuls use
    the weights as the stationary operand and no on-device transposes
    are needed. Matmul operands are fp16 (same PE rate as bf16 — 1
    elem/cell/cycle — but 8x finer mantissa; fp32 operands run at 1/4
    rate via hi/lo splitting). Accumulation is fp32 in PSUM; biases,
    GELU, and the per-token combine weight are applied in fp32.
  - Host scatter-adds each core's [H, C] result back into [T, H].

Measured on trn2 (8 cores, C=1152): ~166 us HW exec time,
output max-abs error ~4e-4 of output scale vs the fp32 reference.

Self-contained: hardcodes the problem shapes (H=768, F=3072, E=8, K=2).
"""

import os
import time

import numpy as np

H = 768
F = 3072
E = 8
K = 2
N_CORES = 8
P = 128
CHUNK = 512  # token-chunk width (fp32 PSUM bank = 512 elems)

PRECISION = os.environ.get("MOE_PRECISION", "fp16")  # "fp16" | "bf16" | "fp32"


# ---------------------------------------------------------------------------
# Bass/Tile device kernel
# ---------------------------------------------------------------------------

def _build_bass(C, Hd=H, Fd=F, precision=None):
    """Build + compile the per-core Bass program for capacity C."""
    from contextlib import ExitStack

    import concourse.bass as bass  # noqa: F401
    import concourse.tile as tile
    from concourse import bacc, mybir
    from concourse._compat import with_exitstack

    precision = precision or PRECISION
    assert C % P == 0 and Hd % P == 0 and Fd % (8 * P) == 0
    FM = Fd // P          # number of 128-row tiles of the F dim
    HK = Hd // P          # contraction tiles for x@W1
    HN = Hd // P          # output row tiles of yT
    f32 = mybir.dt.float32
    mdt = {"bf16": mybir.dt.bfloat16, "fp16": mybir.dt.float16,
           "fp32": f32}[precision]

    chunks = []
    c0 = 0
    while c0 < C:
        w = min(CHUNK, C - c0)
        chunks.append((c0, w))
        c0 += w

    nc = bacc.Bacc("TRN2", target_bir_lowering=False, debug=False,
                   num_devices=N_CORES)
    xgT = nc.dram_tensor("xgt", [Hd, C], mdt, kind="ExternalInput").ap()
    w1 = nc.dram_tensor("w1", [Hd, Fd], mdt, kind="ExternalInput").ap()
    w2 = nc.dram_tensor("w2", [Fd, Hd], mdt, kind="ExternalInput").ap()
    # packed fp32 constants: [b1t | b2c | wtb] along the free dim
    cpk = nc.dram_tensor("cpk", [P, FM + HN + C], f32,
                         kind="ExternalInput").ap()
    y = nc.dram_tensor("y", [Hd, C], f32, kind="ExternalOutput").ap()

    gelu = mybir.ActivationFunctionType.Gelu_apprx_tanh
    ident = mybir.ActivationFunctionType.Identity

    @with_exitstack
    def body(ctx: ExitStack, tc: tile.TileContext):
        const = ctx.enter_context(tc.tile_pool(name="const", bufs=1))
        w1p = ctx.enter_context(tc.tile_pool(name="w1p", bufs=1))
        w2p = ctx.enter_context(tc.tile_pool(name="w2p", bufs=1))
        xp = ctx.enter_context(tc.tile_pool(name="xp", bufs=1))
        hp = ctx.enter_context(tc.tile_pool(name="hp", bufs=1))
        yp = ctx.enter_context(tc.tile_pool(name="yp", bufs=3))
        psAp = ctx.enter_context(tc.tile_pool(name="psA", bufs=2, space="PSUM"))
        psBp = ctx.enter_context(tc.tile_pool(name="psB", bufs=6, space="PSUM"))

        # Everything is resident in SBUF (fp16 weights are small enough).
        # Two HWDGE rings (sync + scalar), each FIFO; both pull from the
        # same ~360 GB/s per-core HBM pipe. The first MM gates only on
        # xg0 + W1 quarter 0, so the whole W1 chain rides the sync ring
        # (quarters land ~2x faster than phase A consumes them), while
        # the scalar ring carries only late-need bulk (q3, b2/wt, W2).
        c00, w0 = chunks[0]
        NW1 = 8                       # W1 pieces (finer => earlier first MM)
        FQ = Fd // NW1
        # Gate items (xg0 + W1 piece 0) ride DIFFERENT rings so they
        # transfer concurrently; b1 is tiny and precedes xg0 on scalar.
        b1s = const.tile([P, FM], f32, name="b1s")
        nc.scalar.dma_start(b1s[:], cpk[:, 0:FM])
        xg0 = xp.tile([P, HK, CHUNK], mdt, tag="xg0", name="xg0")
        nc.scalar.dma_start(
            xg0[:, :, :w0],
            xgT[:, 0:w0].rearrange("(k p) c -> p k c", p=P))
        cps2 = const.tile([P, HN + C], f32, name="cps2")
        b2s = cps2[:, 0:HN]
        wtbs = cps2[:, HN:]
        w1q = []
        for g in range(NW1):
            t = w1p.tile([P, HK, FQ], mdt, tag=f"w1q{g}", name=f"w1q{g}")
            src = w1[:, g * FQ:(g + 1) * FQ].rearrange(
                "(k p) f -> p k f", p=P)
            (nc.sync if g % 2 == 0 else nc.scalar).dma_start(t[:], src)
            w1q.append(t)
        xgf = None
        if C > w0:
            xgf = xp.tile([P, HK, C - w0], mdt, tag="xgf", name="xgf")
            nc.sync.dma_start(
                xgf[:], xgT[:, w0:].rearrange("(k p) c -> p k c", p=P))
        nc.scalar.dma_start(cps2[:], cpk[:, FM:])
        W2G = FM // 2  # 12 row-tiles per grouped DMA
        w2g = []
        for g in range(2):
            t = w2p.tile([P, W2G, Hd], mdt, tag=f"w2g{g}", name=f"w2g{g}")
            src = w2[g * W2G * P:(g + 1) * W2G * P, :].rearrange(
                "(k p) f -> p k f", p=P)
            (nc.sync if g % 2 == 0 else nc.scalar).dma_start(t[:], src)
            w2g.append(t)
        w2t = [w2g[k // W2G][:, k % W2G, :] for k in range(FM)]

        FQT = FQ // P  # fm tiles per W1 quarter
        def w1_tile(hk, fm):
            return w1q[fm // FQT][:, hk, (fm % FQT) * P:(fm % FQT + 1) * P]

        def rhs_x(hk, c0, w):
            if c0 == 0:
                return xg0[:, hk, :w]
            return xgf[:, hk, c0 - w0:c0 - w0 + w]

        for ci, (c0, w) in enumerate(chunks):
            # ---- phase A: hT[f, c] = gelu((x@W1)[c, f] + b1[f]) ----
            hts = [None] * FM
            for fm in range(FM):
                ps = psAp.tile([P, CHUNK], f32, tag="psA", name="psA")
                for hk in range(HK):
                    nc.tensor.matmul(
                        ps[:, :w],
                        lhsT=w1_tile(hk, fm),
                        rhs=rhs_x(hk, c0, w),
                        start=(hk == 0), stop=(hk == HK - 1),
                    )
                ht = hp.tile([P, CHUNK], mdt, tag=f"hts{fm}",
                             name=f"hts{fm}")
                nc.scalar.activation(ht[:, :w], ps[:, :w], gelu,
                                     bias=b1s[:, fm:fm + 1])
                hts[fm] = ht

            # ---- phase B: yT[h, c] = sum_f W2[f, h] * hT[f, c] ----
            for hn in range(HN):
                ps = psBp.tile([P, CHUNK], f32, tag="psB", name="psB")
                for fk in range(FM):
                    nc.tensor.matmul(
                        ps[:, :w],
                        lhsT=w2t[fk][:, hn * P:(hn + 1) * P],
                        rhs=hts[fk][:, :w],
                        start=(fk == 0), stop=(fk == FM - 1),
                    )
                # ---- epilogue: (+b2), (*wt), store ----
                ot = yp.tile([P, CHUNK], f32, tag="yout", name="yout")
                nc.scalar.activation(ot[:, :w], ps[:, :w], ident,
                                     bias=b2s[:, hn:hn + 1])
                ot2 = yp.tile([P, CHUNK], f32, tag="yout2", name="yout2")
                nc.vector.tensor_mul(ot2[:, :w], ot[:, :w],
                                     wtbs[:, c0:c0 + w])
                nc.sync.dma_start(y[hn * P:(hn + 1) * P, c0:c0 + w],
                                  ot2[:, :w])

    with tile.TileContext(nc) as tc:
        body(tc)
    nc.compile()
    return nc


# ---------------------------------------------------------------------------
# Host-side routing + dispatch
# ---------------------------------------------------------------------------

def _route(xf, gate_w):
    """Top-2 router in float64 for a numerically robust top-k set.

    Returns per-expert (token_idx, weight) lists.
    """
    logits = xf.astype(np.float64) @ gate_w.astype(np.float64)  # [T, E]
    top_idx = np.argpartition(logits, E - K, axis=1)[:, E - K:]  # [T, K]
    top_val = np.take_along_axis(logits, top_idx, axis=1)
    m = top_val.max(axis=1, keepdims=True)
    ex = np.exp(top_val - m)
    wts = ex / ex.sum(axis=1, keepdims=True)  # [T, K] float64

    toks, ws = [], []
    for e in range(E):
        mask = top_idx == e  # [T, K]
        rows = np.nonzero(mask.any(axis=1))[0]
        toks.append(rows)
        ws.append(wts[mask].astype(np.float32))
    return toks, ws


def _np_mdt():
    import ml_dtypes
    return {"bf16": ml_dtypes.bfloat16, "fp16": np.float16,
            "fp32": np.float32}[PRECISION]


def _make_in_maps(xf, gate_w, W1, b1, W2, b2):
    toks, ws = _route(xf, gate_w)
    nmax = max(len(t) for t in toks)
    C = max(P, ((nmax + P - 1) // P) * P)
    mdt = _np_mdt()

    W1a = np.asarray(W1, np.float32)
    b1a = np.asarray(b1, np.float32)
    W2a = np.asarray(W2, np.float32)
    b2a = np.asarray(b2, np.float32)
    in_maps = []
    for e in range(E):
        n_e = len(toks[e])
        xgT = np.zeros((H, C), mdt)
        xgT[:, :n_e] = xf[toks[e]].T.astype(mdt)
        wtb = np.zeros((P, C), np.float32)
        wtb[:, :n_e] = ws[e][None, :]
        cpk = np.concatenate([
            b1a[e].reshape(F // P, P).T,
            b2a[e].reshape(H // P, P).T,
            wtb,
        ], axis=1)
        in_maps.append({
            "xgt": xgT,
            "w1": W1a[e].astype(mdt),
            "w2": W2a[e].astype(mdt),
            "cpk": np.ascontiguousarray(cpk),
        })
    return in_maps, toks, C


def _run(inputs, trace=False):
    global PRECISION
    from concourse.bass_utils import run_bass_kernel_spmd

    x, gate_w, W1, b1, W2, b2 = (inputs[k] for k in
                                 ("x", "gate_w", "W1", "b1", "W2", "b2"))
    x = np.asarray(x)
    Bb, S, Hd = x.shape
    assert Hd == H
    T = Bb * S
    xf = np.ascontiguousarray(x.reshape(T, Hd), dtype=np.float32)
    gate_w = np.asarray(gate_w, np.float32)

    # fp16 matmul operands need moderate dynamic range; fall back to
    # bf16 (full fp32 exponent range) if the data is far outside the
    # expected unit-scale regime.
    if PRECISION == "fp16":
        amax = max(float(np.abs(np.asarray(t)).max())
                   for t in (xf, W1, W2))
        if not np.isfinite(amax) or amax > 1e3:
            PRECISION = "bf16"

    in_maps, toks, C = _make_in_maps(xf, gate_w, W1, b1, W2, b2)
    nc = _build_bass(C)

    kwargs = {}
    if trace:
        kwargs = dict(trace=True, trace_cores=list(range(N_CORES)))
    try:
        res = run_bass_kernel_spmd(nc, in_maps, core_ids=list(range(N_CORES)),
                                   **kwargs)
    except Exception:
        # One retry for transient device faults.
        time.sleep(5)
        res = run_bass_kernel_spmd(nc, in_maps, core_ids=list(range(N_CORES)),
                                   **kwargs)
    out = np.zeros((T, H), np.float32)
    for e in range(E):
        n_e = len(toks[e])
        out[toks[e]] += res.results[e]["y"][:, :n_e].T
    return out.reshape(Bb, S, Hd), res


def kernel(x, gate_w, W1, b1, W2, b2):
    out, _ = _run({"x": x, "gate_w": gate_w, "W1": W1, "b1": b1,
                   "W2": W2, "b2": b2})
    return out.astype(np.asarray(x).dtype, copy=False)


# Exposed for test.py: run with profiling, return (output, BassKernelResults)
def kernel_profiled(x, gate_w, W1, b1, W2, b2):
    return _run({"x": x, "gate_w": gate_w, "W1": W1, "b1": b1,
                 "W2": W2, "b2": b2}, trace=True)


# revision 42
# speedup vs baseline: 1.0191x; 1.0191x over previous
"""MoE FFN (top-2 routing, 8 experts) on 8 Trainium2 NeuronCores.

Strategy (expert parallelism, per the sharding hint):
  - Host computes router logits / top-2 / softmax (tiny: T x E) and
    dispatches tokens: expert e's tokens are gathered into a padded
    [H, C] batch for core e (C = common capacity, multiple of 128).
  - Core e runs the dense FFN for its expert on its gathered tokens:
        yT = wt ⊙ ( GELU_tanh(x @ W1 + b1) @ W2 + b2 )^T
    computed fully transposed ([F,C] then [H,C]) so both matmuls use
    the weights as the stationary operand and no on-device transposes
    are needed. Matmul operands are fp16 (same PE rate as bf16 — 1
    elem/cell/cycle — but 8x finer mantissa; fp32 operands run at 1/4
    rate via hi/lo splitting). Accumulation is fp32 in PSUM; biases,
    GELU, and the per-token combine weight are applied in fp32.
  - Host scatter-adds each core's [H, C] result back into [T, H].

Measured on trn2 (8 cores, C=1152): ~166 us HW exec time,
output max-abs error ~4e-4 of output scale vs the fp32 reference.

Self-contained: hardcodes the problem shapes (H=768, F=3072, E=8, K=2).
"""

import os
import time

import numpy as np

H = 768
F = 3072
E = 8
K = 2
N_CORES = 8
P = 128
CHUNK = 512  # token-chunk width (fp32 PSUM bank = 512 elems)

PRECISION = os.environ.get("MOE_PRECISION", "fp16")  # "fp16" | "bf16" | "fp32"


# ---------------------------------------------------------------------------
# Bass/Tile device kernel
# ---------------------------------------------------------------------------

def _build_bass(C, Hd=H, Fd=F, precision=None):
    """Build + compile the per-core Bass program for capacity C."""
    from contextlib import ExitStack

    import concourse.bass as bass  # noqa: F401
    import concourse.tile as tile
    from concourse import bacc, mybir
    from concourse._compat import with_exitstack

    precision = precision or PRECISION
    assert C % P == 0 and Hd % P == 0 and Fd % (8 * P) == 0
    FM = Fd // P          # number of 128-row tiles of the F dim
    HK = Hd // P          # contraction tiles for x@W1
    HN = Hd // P          # output row tiles of yT
    f32 = mybir.dt.float32
    mdt = {"bf16": mybir.dt.bfloat16, "fp16": mybir.dt.float16,
           "fp32": f32}[precision]

    chunks = []
    c0 = 0
    while c0 < C:
        w = min(CHUNK, C - c0)
        chunks.append((c0, w))
        c0 += w

    nc = bacc.Bacc("TRN2", target_bir_lowering=False, debug=False,
                   num_devices=N_CORES)
    xgT = nc.dram_tensor("xgt", [Hd, C], mdt, kind="ExternalInput").ap()
    w1 = nc.dram_tensor("w1", [Hd, Fd], mdt, kind="ExternalInput").ap()
    w2 = nc.dram_tensor("w2", [Fd, Hd], mdt, kind="ExternalInput").ap()
    # packed fp32 constants: [b1t | b2c | wtb] along the free dim
    cpk = nc.dram_tensor("cpk", [P, FM + HN + C], f32,
                         kind="ExternalInput").ap()
    y = nc.dram_tensor("y", [Hd, C], f32, kind="ExternalOutput").ap()

    gelu = mybir.ActivationFunctionType.Gelu_apprx_tanh
    ident = mybir.ActivationFunctionType.Identity

    @with_exitstack
    def body(ctx: ExitStack, tc: tile.TileContext):
        const = ctx.enter_context(tc.tile_pool(name="const", bufs=1))
        w1p = ctx.enter_context(tc.tile_pool(name="w1p", bufs=1))
        w2p = ctx.enter_context(tc.tile_pool(name="w2p", bufs=1))
        xp = ctx.enter_context(tc.tile_pool(name="xp", bufs=1))
        hp = ctx.enter_context(tc.tile_pool(name="hp", bufs=1))
        yp = ctx.enter_context(tc.tile_pool(name="yp", bufs=3))
        psAp = ctx.enter_context(tc.tile_pool(name="psA", bufs=2, space="PSUM"))
        psBp = ctx.enter_context(tc.tile_pool(name="psB", bufs=6, space="PSUM"))

        # Everything is resident in SBUF (fp16 weights are small enough).
        # Two HWDGE rings (sync + scalar), each FIFO; both pull from the
        # same ~360 GB/s per-core HBM pipe. The first MM gates only on
        # xg0 + W1 quarter 0, so the whole W1 chain rides the sync ring
        # (quarters land ~2x faster than phase A consumes them), while
        # the scalar ring carries only late-need bulk (q3, b2/wt, W2).
        c00, w0 = chunks[0]
        NW1 = 8                       # W1 pieces (finer => earlier first MM)
        FQ = Fd // NW1
        xg0 = xp.tile([P, HK, CHUNK], mdt, tag="xg0", name="xg0")
        nc.sync.dma_start(
            xg0[:, :, :w0],
            xgT[:, 0:w0].rearrange("(k p) c -> p k c", p=P))
        # b1 is needed by the first activation (~stage 1); b2/wt only by
        # the first epilogue (~stage 3).
        b1s = const.tile([P, FM], f32, name="b1s")
        nc.scalar.dma_start(b1s[:], cpk[:, 0:FM])
        cps2 = const.tile([P, HN + C], f32, name="cps2")
        b2s = cps2[:, 0:HN]
        wtbs = cps2[:, HN:]
        w1q = []
        for g in range(NW1):
            t = w1p.tile([P, HK, FQ], mdt, tag=f"w1q{g}", name=f"w1q{g}")
            src = w1[:, g * FQ:(g + 1) * FQ].rearrange(
                "(k p) f -> p k f", p=P)
            (nc.sync if g % 2 == 0 else nc.scalar).dma_start(t[:], src)
            w1q.append(t)
        xgf = None
        if C > w0:
            xgf = xp.tile([P, HK, C - w0], mdt, tag="xgf", name="xgf")
            nc.sync.dma_start(
                xgf[:], xgT[:, w0:].rearrange("(k p) c -> p k c", p=P))
        nc.scalar.dma_start(cps2[:], cpk[:, FM:])
        W2G = FM // 2  # 12 row-tiles per grouped DMA
        w2g = []
        for g in range(2):
            t = w2p.tile([P, W2G, Hd], mdt, tag=f"w2g{g}", name=f"w2g{g}")
            src = w2[g * W2G * P:(g + 1) * W2G * P, :].rearrange(
                "(k p) f -> p k f", p=P)
            (nc.sync if g % 2 == 0 else nc.scalar).dma_start(t[:], src)
            w2g.append(t)
        w2t = [w2g[k // W2G][:, k % W2G, :] for k in range(FM)]

        FQT = FQ // P  # fm tiles per W1 quarter
        def w1_tile(hk, fm):
            return w1q[fm // FQT][:, hk, (fm % FQT) * P:(fm % FQT + 1) * P]

        def rhs_x(hk, c0, w):
            if c0 == 0:
                return xg0[:, hk, :w]
            return xgf[:, hk, c0 - w0:c0 - w0 + w]

        for ci, (c0, w) in enumerate(chunks):
            # ---- phase A: hT[f, c] = gelu((x@W1)[c, f] + b1[f]) ----
            hts = [None] * FM
            for fm in range(FM):
                ps = psAp.tile([P, CHUNK], f32, tag="psA", name="psA")
                for hk in range(HK):
                    nc.tensor.matmul(
                        ps[:, :w],
                        lhsT=w1_tile(hk, fm),
                        rhs=rhs_x(hk, c0, w),
                        start=(hk == 0), stop=(hk == HK - 1),
                    )
                ht = hp.tile([P, CHUNK], mdt, tag=f"hts{fm}",
                             name=f"hts{fm}")
                nc.scalar.activation(ht[:, :w], ps[:, :w], gelu,
                                     bias=b1s[:, fm:fm + 1])
                hts[fm] = ht

            # ---- phase B: yT[h, c] = sum_f W2[f, h] * hT[f, c] ----
            for hn in range(HN):
                ps = psBp.tile([P, CHUNK], f32, tag="psB", name="psB")
                for fk in range(FM):
                    nc.tensor.matmul(
                        ps[:, :w],
                        lhsT=w2t[fk][:, hn * P:(hn + 1) * P],
                        rhs=hts[fk][:, :w],
                        start=(fk == 0), stop=(fk == FM - 1),
                    )
                # ---- epilogue: (+b2), (*wt), store ----
                ot = yp.tile([P, CHUNK], f32, tag="yout", name="yout")
                nc.scalar.activation(ot[:, :w], ps[:, :w], ident,
                                     bias=b2s[:, hn:hn + 1])
                ot2 = yp.tile([P, CHUNK], f32, tag="yout2", name="yout2")
                nc.vector.tensor_mul(ot2[:, :w], ot[:, :w],
                                     wtbs[:, c0:c0 + w])
                nc.sync.dma_start(y[hn * P:(hn + 1) * P, c0:c0 + w],
                                  ot2[:, :w])

    with tile.TileContext(nc) as tc:
        body(tc)
    nc.compile()
    return nc


# ---------------------------------------------------------------------------
# Host-side routing + dispatch
# ---------------------------------------------------------------------------

def _route(xf, gate_w):
    """Top-2 router in float64 for a numerically robust top-k set.

    Returns per-expert (token_idx, weight) lists.
    """
    logits = xf.astype(np.float64) @ gate_w.astype(np.float64)  # [T, E]
    top_idx = np.argpartition(logits, E - K, axis=1)[:, E - K:]  # [T, K]
    top_val = np.take_along_axis(logits, top_idx, axis=1)
    m = top_val.max(axis=1, keepdims=True)
    ex = np.exp(top_val - m)
    wts = ex / ex.sum(axis=1, keepdims=True)  # [T, K] float64

    toks, ws = [], []
    for e in range(E):
        mask = top_idx == e  # [T, K]
        rows = np.nonzero(mask.any(axis=1))[0]
        toks.append(rows)
        ws.append(wts[mask].astype(np.float32))
    return toks, ws


def _np_mdt():
    import ml_dtypes
    return {"bf16": ml_dtypes.bfloat16, "fp16": np.float16,
            "fp32": np.float32}[PRECISION]


def _make_in_maps(xf, gate_w, W1, b1, W2, b2):
    toks, ws = _route(xf, gate_w)
    nmax = max(len(t) for t in toks)
    C = max(P, ((nmax + P - 1) // P) * P)
    mdt = _np_mdt()

    W1a = np.asarray(W1, np.float32)
    b1a = np.asarray(b1, np.float32)
    W2a = np.asarray(W2, np.float32)
    b2a = np.asarray(b2, np.float32)
    in_maps = []
    for e in range(E):
        n_e = len(toks[e])
        xgT = np.zeros((H, C), mdt)
        xgT[:, :n_e] = xf[toks[e]].T.astype(mdt)
        wtb = np.zeros((P, C), np.float32)
        wtb[:, :n_e] = ws[e][None, :]
        cpk = np.concatenate([
            b1a[e].reshape(F // P, P).T,
            b2a[e].reshape(H // P, P).T,
            wtb,
        ], axis=1)
        in_maps.append({
            "xgt": xgT,
            "w1": W1a[e].astype(mdt),
            "w2": W2a[e].astype(mdt),
            "cpk": np.ascontiguousarray(cpk),
        })
    return in_maps, toks, C


def _run(inputs, trace=False):
    global PRECISION
    from concourse.bass_utils import run_bass_kernel_spmd

    x, gate_w, W1, b1, W2, b2 = (inputs[k] for k in
                                 ("x", "gate_w", "W1", "b1", "W2", "b2"))
    x = np.asarray(x)
    Bb, S, Hd = x.shape
    assert Hd == H
    T = Bb * S
    xf = np.ascontiguousarray(x.reshape(T, Hd), dtype=np.float32)
    gate_w = np.asarray(gate_w, np.float32)

    # fp16 matmul operands need moderate dynamic range; fall back to
    # bf16 (full fp32 exponent range) if the data is far outside the
    # expected unit-scale regime.
    if PRECISION == "fp16":
        amax = max(float(np.abs(np.asarray(t)).max())
                   for t in (xf, W1, W2))
        if not np.isfinite(amax) or amax > 1e3:
            PRECISION = "bf16"

    in_maps, toks, C = _make_in_maps(xf, gate_w, W1, b1, W2, b2)
    nc = _build_bass(C)

    kwargs = {}
    if trace:
        kwargs = dict(trace=True, trace_cores=list(range(N_CORES)))
    try:
        res = run_bass_kernel_spmd(nc, in_maps, core_ids=list(range(N_CORES)),
                                   **kwargs)
    except Exception:
        # One retry for transient device faults.
        time.sleep(5)
        res = run_bass_kernel_spmd(nc, in_maps, core_ids=list(range(N_CORES)),
                                   **kwargs)
    out = np.zeros((T, H), np.float32)
    for e in range(E):
        n_e = len(toks[e])
        out[toks[e]] += res.results[e]["y"][:, :n_e].T
    return out.reshape(Bb, S, Hd), res


def kernel(x, gate_w, W1, b1, W2, b2):
    out, _ = _run({"x": x, "gate_w": gate_w, "W1": W1, "b1": b1,
                   "W2": W2, "b2": b2})
    return out.astype(np.asarray(x).dtype, copy=False)


# Exposed for test.py: run with profiling, return (output, BassKernelResults)
def kernel_profiled(x, gate_w, W1, b1, W2, b2):
    return _run({"x": x, "gate_w": gate_w, "W1": W1, "b1": b1,
                 "W2": W2, "b2": b2}, trace=True)
